# revision 6
# baseline (speedup 1.0000x reference)
"""LCAM (sparse_attention) Trainium2 Bass kernel.

Math (reference collapsed):
  softmax over a size-1 axis is exactly 1.0, so the axial "attention" paths
  reduce to out_h = bn2(2*v_h) + xh2 and out_w = bn2(2*v_w) + xw2.

  Per batch b, channel c  (H = W = 64):
    x_h_sum[c,i] = sum_j x[c,i,j];  x_w_sum[c,j] = sum_i x[c,i,j]
    x_avg[c] = sum(x)/4096;  x_max[c] = max(x)
    a = relu(w_conv1 @ [x_max | x_avg] + b_conv1)          # [64, 2]
    gate = sigmoid(w_conv @ concat(a_max, a_avg) + b_conv) # [512]
    s = 1 + gate
    z = w1bn @ [x_h_sum | x_w_sum] + b1bn                  # [64, 128] (1/64 + bn1 folded)
    y2 = z * min(relu(z+3), 6) / 6                         # hswish
    xhw2 = w_conv0 @ y2 + b_conv0                          # [512, 128]
    out_hw = wv' @ xhw2 + bv' + xhw2                       # wv' = 2*s2*w_v (bn2 folded)
    out[c, i*64+j] = out_h[c,i]*out_w[c,j] + x[c,i*64+j]*s[c]

Engine split per [128, 4096] tile (8 tiles/core, 2 batches x 4 channel tiles):
  DVE:    3 reductions (x_h, x_w strided, max)
  GPSIMD: outer product via broadcast tensor_tensor
  PE:     diag(s) @ x  +  I @ outer  accumulated in PSUM
  ACT:    PSUM -> SBUF evacuation (in-place over x tile)
  DMA:    16 MiB in + 16 MiB out per core (roofline ~94 us)
"""

import numpy as np
from contextlib import ExitStack

import concourse.bass as bass
import concourse.bacc as bacc
import concourse.tile as tile
import concourse.mybir as mybir

F32 = mybir.dt.float32
AF = mybir.ActivationFunctionType
ALU = mybir.AluOpType
AX = mybir.AxisListType

B, C, H, W = 16, 512, 64, 64
HW = H * W
EPS = 1e-5
NCORES = 8
NB = B // NCORES          # 2 batches per core
CT = C // 128             # 4 channel tiles
QUARTER = 1024            # pass-C PSUM chunk (2 banks)


def build_program():
    nc = bacc.Bacc(trn_type="TRN2", name="lcam")

    x_d = nc.dram_tensor("x_sh", [NB, C, H, W], F32, kind="ExternalInput")
    w1T_d = nc.dram_tensor("w1T", [C, 64], F32, kind="ExternalInput")
    w1bnT_d = nc.dram_tensor("w1bnT", [C, 64], F32, kind="ExternalInput")
    wcmT_d = nc.dram_tensor("wcmT", [64, C], F32, kind="ExternalInput")
    wcaT_d = nc.dram_tensor("wcaT", [64, C], F32, kind="ExternalInput")
    w0T_d = nc.dram_tensor("w0T", [64, C], F32, kind="ExternalInput")
    wvT_d = nc.dram_tensor("wvT", [C, C], F32, kind="ExternalInput")
    b128_d = nc.dram_tensor("b128", [128, 12], F32, kind="ExternalInput")
    b64_d = nc.dram_tensor("b64", [64, 2], F32, kind="ExternalInput")
    ident_d = nc.dram_tensor("ident", [128, 128], F32, kind="ExternalInput")
    y_d = nc.dram_tensor("y_sh", [NB, C, H, W], F32, kind="ExternalOutput")

    with ExitStack() as ctx:
        tc = ctx.enter_context(tile.TileContext(nc))
        wp = ctx.enter_context(tc.tile_pool(name="wp", bufs=1))
        xp = ctx.enter_context(tc.tile_pool(name="xp", bufs=8))
        redp = ctx.enter_context(tc.tile_pool(name="redp", bufs=8))
        smallp = ctx.enter_context(tc.tile_pool(name="smallp", bufs=8))
        hwp = ctx.enter_context(tc.tile_pool(name="hwp", bufs=8))
        diagp = ctx.enter_context(tc.tile_pool(name="diagp", bufs=8))
        outerp = ctx.enter_context(tc.tile_pool(name="outerp", bufs=2))
        psC = ctx.enter_context(tc.tile_pool(name="psC", bufs=2, space="PSUM"))
        psM = ctx.enter_context(tc.tile_pool(name="psM", bufs=4, space="PSUM"))

        # ---- load weights / constants ----
        w1T = []
        w1bnT = []
        wvT = []
        for ct in range(CT):
            t1 = wp.tile([128, 64], F32, name=f"w1T_{ct}")
            nc.sync.dma_start(out=t1, in_=w1T_d.ap()[ct * 128:(ct + 1) * 128, :])
            w1T.append(t1)
            t2 = wp.tile([128, 64], F32, name=f"w1bnT_{ct}")
            nc.sync.dma_start(out=t2, in_=w1bnT_d.ap()[ct * 128:(ct + 1) * 128, :])
            w1bnT.append(t2)
            t3 = wp.tile([128, C], F32, name=f"wvT_{ct}")
            nc.sync.dma_start(out=t3, in_=wvT_d.ap()[ct * 128:(ct + 1) * 128, :])
            wvT.append(t3)
        wcmT = wp.tile([64, C], F32)
        nc.sync.dma_start(out=wcmT, in_=wcmT_d.ap())
        wcaT = wp.tile([64, C], F32)
        nc.sync.dma_start(out=wcaT, in_=wcaT_d.ap())
        w0T = wp.tile([64, C], F32)
        nc.sync.dma_start(out=w0T, in_=w0T_d.ap())
        b128 = wp.tile([128, 12], F32)
        nc.sync.dma_start(out=b128, in_=b128_d.ap())
        b64 = wp.tile([64, 2], F32)
        nc.sync.dma_start(out=b64, in_=b64_d.ap())
        ident = wp.tile([128, 128], F32)
        nc.sync.dma_start(out=ident, in_=ident_d.ap())
        three = wp.tile([64, 1], F32)
        nc.vector.memset(three, 3.0)

        for b in range(NB):
            xt = []
            red = []
            rhs_se = []
            # ---- pass A: load + reductions ----
            for ct in range(CT):
                x_tile = xp.tile([128, HW], F32, tag="x")
                nc.sync.dma_start(
                    out=x_tile,
                    in_=x_d.ap()[b, ct * 128:(ct + 1) * 128].rearrange("c h w -> c (h w)"),
                )
                xt.append(x_tile)

                xhw = redp.tile([128, 128], F32, tag="red")
                nc.vector.reduce_sum(
                    out=xhw[:, 0:64],
                    in_=x_tile.rearrange("p (i j) -> p i j", j=W),
                    axis=AX.X,
                )
                nc.vector.reduce_sum(
                    out=xhw[:, 64:128],
                    in_=x_tile.rearrange("p (i j) -> p j i", j=W),
                    axis=AX.X,
                )
                red.append(xhw)

                se = smallp.tile([128, 2], F32, tag="se")
                nc.vector.reduce_max(out=se[:, 0:1], in_=x_tile, axis=AX.X)
                rsum = smallp.tile([128, 1], F32, tag="rsum")
                nc.vector.reduce_sum(out=rsum, in_=xhw[:, 0:64], axis=AX.X)
                nc.vector.tensor_scalar_mul(se[:, 1:2], rsum, 1.0 / HW)
                rhs_se.append(se)

            # ---- middle: SE gate ----
            psa = psM.tile([64, 2], F32, tag="mm128")
            for ct in range(CT):
                nc.tensor.matmul(psa, w1T[ct], rhs_se[ct],
                                 start=(ct == 0), stop=(ct == CT - 1))
            a_sb = smallp.tile([64, 2], F32, tag="a_sb")
            nc.scalar.activation(a_sb, psa, AF.Relu, bias=b64[:, 0:1], scale=1.0)

            diag_s = []
            for ct in range(CT):
                psg = psM.tile([128, 1], F32, tag="mm128")
                nc.tensor.matmul(psg, wcmT[:, ct * 128:(ct + 1) * 128], a_sb[:, 0:1],
                                 start=True, stop=False)
                nc.tensor.matmul(psg, wcaT[:, ct * 128:(ct + 1) * 128], a_sb[:, 1:2],
                                 start=False, stop=True)
                s_ct = smallp.tile([128, 1], F32, tag="s_ct")
                nc.scalar.activation(s_ct, psg, AF.Sigmoid,
                                     bias=b128[:, ct:ct + 1], scale=1.0)
                nc.vector.tensor_scalar_add(s_ct, s_ct, 1.0)
                dg = diagp.tile([128, 128], F32, tag="diag")
                nc.vector.tensor_scalar_mul(dg, ident, s_ct)
                diag_s.append(dg)

            # ---- middle: coordinate branch ----
            psy = psM.tile([64, 128], F32, tag="mm128")
            for ct in range(CT):
                nc.tensor.matmul(psy, w1bnT[ct], red[ct],
                                 start=(ct == 0), stop=(ct == CT - 1))
            z_sb = smallp.tile([64, 128], F32, tag="z_sb")
            nc.scalar.activation(z_sb, psy, AF.Identity, bias=b64[:, 1:2], scale=1.0)
            t_sb = smallp.tile([64, 128], F32, tag="t_sb")
            nc.scalar.activation(t_sb, z_sb, AF.Relu, bias=three, scale=1.0)
            nc.vector.tensor_scalar_min(t_sb, t_sb, 6.0)
            y2 = smallp.tile([64, 128], F32, tag="y2")
            nc.vector.scalar_tensor_tensor(
                out=y2, in0=t_sb, scalar=1.0 / 6.0, in1=z_sb,
                op0=ALU.mult, op1=ALU.mult)

            xhw2 = []
            for ct in range(CT):
                psb = psM.tile([128, 128], F32, tag="mm128")
                nc.tensor.matmul(psb, w0T[:, ct * 128:(ct + 1) * 128], y2,
                                 start=True, stop=True)
                xh2 = hwp.tile([128, 128], F32, tag="xhw2")
                nc.scalar.activation(xh2, psb, AF.Identity,
                                     bias=b128[:, 4 + ct:5 + ct], scale=1.0)
                xhw2.append(xh2)

            out_hw = []
            for ct in range(CT):
                psv = psM.tile([128, 128], F32, tag="mm128")
                for kct in range(CT):
                    nc.tensor.matmul(psv, wvT[kct][:, ct * 128:(ct + 1) * 128],
                                     xhw2[kct], start=(kct == 0), stop=(kct == CT - 1))
                ohw = hwp.tile([128, 128], F32, tag="ohw")
                nc.vector.scalar_tensor_tensor(
                    out=ohw, in0=psv, scalar=b128[:, 8 + ct:9 + ct], in1=xhw2[ct],
                    op0=ALU.add, op1=ALU.add)
                out_hw.append(ohw)

            # ---- pass C ----
            for ct in range(CT):
                x_tile = xt[ct]
                for half in range(2):
                    outer = outerp.tile([128, 2048], F32, tag="outer")
                    oh = (out_hw[ct][:, half * 32:(half + 1) * 32]
                          .unsqueeze(2).broadcast_to([128, 32, 64]))
                    ow = (out_hw[ct][:, 64:128]
                          .unsqueeze(1).broadcast_to([128, 32, 64]))
                    nc.gpsimd.tensor_tensor(
                        out=outer.rearrange("p (i j) -> p i j", j=W),
                        in0=oh, in1=ow, op=ALU.mult)
                    for q in range(2):
                        ps = psC.tile([128, QUARTER], F32, tag="psC")
                        base = half * 2048 + q * QUARTER
                        for h in range(2):
                            sl = slice(base + h * 512, base + (h + 1) * 512)
                            osl = slice(q * QUARTER + h * 512, q * QUARTER + (h + 1) * 512)
                            nc.tensor.matmul(ps[:, h * 512:(h + 1) * 512],
                                             diag_s[ct], x_tile[:, sl],
                                             start=True, stop=False)
                            nc.tensor.matmul(ps[:, h * 512:(h + 1) * 512],
                                             ident, outer[:, osl],
                                             start=False, stop=True)
                        nc.scalar.activation(
                            x_tile[:, base:base + QUARTER], ps, AF.Copy, scale=1.0)
                nc.sync.dma_start(
                    out=y_d.ap()[b, ct * 128:(ct + 1) * 128].rearrange("c h w -> c (h w)"),
                    in_=x_tile)
    nc.compile()
    return nc


def host_inputs(x, w_conv1, b_conv1, w_conv, b_conv, bn1_g, bn1_b, bn1_m, bn1_v,
                w_conv0, b_conv0, w_v, b_v, bn2_g, bn2_b, bn2_m, bn2_v):
    f = np.float32
    s1 = (bn1_g / np.sqrt(bn1_v + EPS)).astype(f)
    t1 = (bn1_b - bn1_m * s1).astype(f)
    s2 = (bn2_g / np.sqrt(bn2_v + EPS)).astype(f)
    t2 = (bn2_b - bn2_m * s2).astype(f)

    w1T = np.ascontiguousarray(w_conv1.T, dtype=f)                      # [512, 64]
    w1bnT = np.ascontiguousarray(((s1[:, None] / 64.0) * w_conv1).T, dtype=f)
    wcmT = np.ascontiguousarray(w_conv[:, :64].T, dtype=f)              # [64, 512]
    wcaT = np.ascontiguousarray(w_conv[:, 64:].T, dtype=f)
    w0T = np.ascontiguousarray(w_conv0.T, dtype=f)                      # [64, 512]
    wvT = np.ascontiguousarray((2.0 * s2[:, None] * w_v).T, dtype=f)    # [512, 512]
    bv2 = (2.0 * s2 * b_v + t2).astype(f)
    b1bn = (s1 * b_conv1 + t1).astype(f)

    b128 = np.zeros((128, 12), dtype=f)
    for ct in range(CT):
        b128[:, ct] = b_conv[ct * 128:(ct + 1) * 128]
        b128[:, 4 + ct] = b_conv0[ct * 128:(ct + 1) * 128]
        b128[:, 8 + ct] = bv2[ct * 128:(ct + 1) * 128]
    b64 = np.zeros((64, 2), dtype=f)
    b64[:, 0] = b_conv1
    b64[:, 1] = b1bn
    ident = np.eye(128, dtype=f)

    common = dict(w1T=w1T, w1bnT=w1bnT, wcmT=wcmT, wcaT=wcaT, w0T=w0T,
                  wvT=wvT, b128=b128, b64=b64, ident=ident)
    in_maps = []
    for i in range(NCORES):
        m = dict(common)
        m["x_sh"] = np.ascontiguousarray(x[i * NB:(i + 1) * NB], dtype=f)
        in_maps.append(m)
    return in_maps


_NC = None


def kernel(**inputs):
    global _NC
    from concourse.bass_utils import run_bass_kernel_spmd

    used = dict(inputs)
    # unused by the collapsed math: q/k/conv4 feed only a size-1 softmax (==1)
    for k in ("w_q", "b_q", "w_k", "b_k", "w_conv4", "b_conv4"):
        used.pop(k, None)
    in_maps = host_inputs(**{k: np.asarray(v) for k, v in used.items()})

    if _NC is None:
        _NC = build_program()
    res = run_bass_kernel_spmd(_NC, in_maps, core_ids=list(range(NCORES)))
    out = np.concatenate([r["y_sh"] for r in res.results], axis=0)
    return out.astype(np.float32)


if __name__ == "__main__":
    nc = build_program()
    print("built ok:", len(nc.m.functions[0].blocks))
